# revision 1
# baseline (speedup 1.0000x reference)
"""Multi-head causal attention (B=8, S=1024, D=768, H=12) on 8 trn2 NeuronCores.

Strategy: data-parallel over batch (one batch element per core, no collectives).

Per-core dataflow (all matmuls fp32r except A@V in bf16):
  - host passes x^T, so Q^T/K^T come from a transposed projection
    (W stationary, x^T moving) and V from a natural projection
    (x^T stationary, W_v moving) -> no on-device transposes at all.
  - attention computed as S^T[k,q] = K @ Q^T per head, with two heads packed
    into the 128-row PE array via row tiling (dh=64).
  - softmax: exp on ScalarE straight out of PSUM (scale 1/8 folded into W_q
    host-side, no max-subtraction needed for these magnitudes); causal mask
    applied as a bf16 0/1 multiply on the few diagonal-crossing blocks;
    the denominator comes for free as row 64 of the A@V matmul by appending
    a ones column to V; division is folded into the PSUM->SBUF copy of the
    A@V result (reciprocal + DMA partition-broadcast via a DRAM scratch).
  - causal block-skip everywhere (upper-triangular blocks never computed).
"""
import sys

if "/opt/trn_rl_repo" not in sys.path:
    sys.path.insert(0, "/opt/trn_rl_repo")

import numpy as np

B, S, D, H = 8, 1024, 768, 12
DH = 64          # head dim
NC_ = 8          # cores
NT = D // 128    # 6 chunks of 128 along D
ST = S // 128    # 8 tiles of 128 along S
QC = S // 512    # 2 chunks of 512 along queries
VPW = H * (DH + 1)  # 780: v' row width (12 heads x (64 + ones col))

_compiled = None


def _build_masks():
    # mask[i, t, j] = 1 if (128*t + i) <= j else 0  (keep condition), bf16
    import ml_dtypes

    i = np.arange(128)[:, None, None]
    t = np.arange(4)[None, :, None]
    j = np.arange(512)[None, None, :]
    m = ((128 * t + i) <= j).astype(np.float32)
    return m.astype(ml_dtypes.bfloat16)


def _build_nc():
    import concourse.bass as bass
    import concourse.mybir as mybir
    import concourse.tile as tile
    from concourse import bacc

    F32 = mybir.dt.float32
    F32R = mybir.dt.float32r
    BF16 = mybir.dt.bfloat16
    AF = mybir.ActivationFunctionType
    MULT = mybir.AluOpType.mult

    nc = bacc.Bacc("TRN2", target_bir_lowering=False, debug=False)

    xT_d = nc.dram_tensor("xT", [D, S], F32, kind="ExternalInput")
    wq_d = nc.dram_tensor("wq", [D, D], F32, kind="ExternalInput")
    wk_d = nc.dram_tensor("wk", [D, D], F32, kind="ExternalInput")
    wv_d = nc.dram_tensor("wv", [D, D], F32, kind="ExternalInput")
    wp_d = nc.dram_tensor("wp", [D, D], F32, kind="ExternalInput")
    mask_d = nc.dram_tensor("masks", [128, 4, 512], BF16, kind="ExternalInput")
    y_d = nc.dram_tensor("y", [S, D], F32, kind="ExternalOutput")
    recip_d = nc.dram_tensor("recip_scratch", [H, QC, 512], F32)

    with tile.TileContext(nc) as tc:
        with (
            tc.tile_pool(name="static", bufs=1) as static,
            tc.tile_pool(name="w", bufs=14) as wpool,
            tc.tile_pool(name="pt", bufs=10) as ptpool,
            tc.tile_pool(name="small", bufs=6) as small,
            tc.tile_pool(name="y", bufs=3) as ypool,
            tc.tile_pool(name="psb", bufs=3, space="PSUM") as psb,
            tc.tile_pool(name="pso", bufs=2, space="PSUM") as pso,
        ):
            # ---- persistent SBUF ----
            xT = static.tile([128, NT, S], F32R)
            qT = static.tile([128, NT, S], F32R)
            kT = static.tile([128, NT, S], F32R)
            vp = static.tile([128, ST, VPW], BF16)
            outT = static.tile([128, NT, S], F32R)
            msk = static.tile([128, 4, 512], BF16)

            for dc in range(NT):
                nc.sync.dma_start(xT[:, dc, :], xT_d[128 * dc:128 * (dc + 1), :].bitcast(F32R))
            nc.sync.dma_start(msk[:], mask_d[:])
            nc.vector.memset(vp[:], 1.0)

            # ---- stage C: v' = x @ W_v (natural layout) + ones cols ----
            wv_t = []
            for dc in range(NT):
                w = wpool.tile([128, D], F32R, tag="w")
                nc.sync.dma_start(w[:], wv_d[128 * dc:128 * (dc + 1), :].bitcast(F32R))
                wv_t.append(w)
            for st in range(ST):
                ps = psb.tile([128, 1024], F32, tag="big")
                for dc in range(NT):
                    nc.tensor.matmul(
                        ps[:, 0:512], xT[:, dc, 128 * st:128 * (st + 1)],
                        wv_t[dc][:, 0:512], start=(dc == 0), stop=(dc == NT - 1))
                for dc in range(NT):
                    nc.tensor.matmul(
                        ps[:, 512:768], xT[:, dc, 128 * st:128 * (st + 1)],
                        wv_t[dc][:, 512:768], start=(dc == 0), stop=(dc == NT - 1))
                # scatter heads into v' (strided dest, ones cols preserved)
                dst = vp[:, st, :].rearrange("p (h e) -> p h e", e=DH + 1)
                nc.vector.tensor_copy(
                    out=dst[:, 0:8, 0:DH],
                    in_=ps[:, 0:512].rearrange("p (h d) -> p h d", d=DH))
                nc.vector.tensor_copy(
                    out=dst[:, 8:12, 0:DH],
                    in_=ps[:, 512:768].rearrange("p (h d) -> p h d", d=DH))

            # ---- interleaved: per head-pair hp: project qT/kT chunk, attention ----
            def project_chunk(w_tiles, nt, dst):
                # dst[:, nt, :] = (W[:, 128nt:128nt+128]).T @ xT   -> [128, 1024]
                ps = psb.tile([128, 1024], F32, tag="big")
                for dc in range(NT):
                    for sc in range(2):
                        nc.tensor.matmul(
                            ps[:, 512 * sc:512 * (sc + 1)],
                            w_tiles[dc][:, 128 * nt:128 * (nt + 1)],
                            xT[:, dc, 512 * sc:512 * (sc + 1)],
                            start=(dc == 0), stop=(dc == NT - 1))
                nc.vector.tensor_copy(out=dst[:, nt, :], in_=ps[:])

            wq_t, wk_t = [], []
            for dc in range(NT):
                w = wpool.tile([128, D], F32R, tag="w")
                nc.sync.dma_start(w[:], wq_d[128 * dc:128 * (dc + 1), :].bitcast(F32R))
                wq_t.append(w)
            for dc in range(NT):
                w = wpool.tile([128, D], F32R, tag="w")
                nc.sync.dma_start(w[:], wk_d[128 * dc:128 * (dc + 1), :].bitcast(F32R))
                wk_t.append(w)

            for hp in range(NT):  # head pair: heads 2hp (rows 0:64), 2hp+1 (rows 64:128)
                project_chunk(wq_t, hp, qT)
                project_chunk(wk_t, hp, kT)

                for qc in range(QC):
                    K = 4 * (qc + 1)  # causal: kc in [0, K)
                    pts = {0: [], 1: []}  # head_half -> list of pt tiles (kc pairs)
                    for kp in range(K // 2):
                        for hh in range(2):
                            rows = slice(64 * hh, 64 * (hh + 1))
                            ps = psb.tile([128, 1024], F32, tag="big")
                            for j in range(2):
                                kc = 2 * kp + j
                                nc.tensor.matmul(
                                    ps[:, 512 * j:512 * (j + 1)],
                                    kT[rows, hp, 128 * kc:128 * (kc + 1)],
                                    qT[rows, hp, 512 * qc:512 * (qc + 1)],
                                    start=True, stop=True,
                                    tile_position=(64 * hh, 0))
                            pt = ptpool.tile([128, 1024], BF16, tag="pt")
                            nc.scalar.activation(pt[:], ps[:], AF.Exp)
                            for j in range(2):
                                kc = 2 * kp + j
                                t = kc - 4 * qc
                                if 0 <= t <= 3:
                                    half = pt[:, 512 * j:512 * (j + 1)]
                                    nc.vector.tensor_tensor(half, half, msk[:, t, :], MULT)
                            pts[hh].append(pt)

                    for hh in range(2):
                        h = 2 * hp + hh
                        rows = slice(64 * hh, 64 * (hh + 1))
                        po = pso.tile([65, 512], F32, tag="po")
                        for kc in range(K):
                            nc.tensor.matmul(
                                po[:],
                                vp[:, kc, 65 * h:65 * (h + 1)],
                                pts[hh][kc // 2][:, 512 * (kc % 2):512 * (kc % 2 + 1)],
                                start=(kc == 0), stop=(kc == K - 1))
                        # denominator -> reciprocal -> broadcast -> normalize
                        rc = small.tile([1, 512], F32, tag="rc")
                        nc.vector.reciprocal(rc[:], po[64:65, :])
                        nc.sync.dma_start(recip_d[h, qc, :], rc[:])
                        rb = small.tile([64, 512], F32, tag="rb")
                        sl = recip_d[h, qc, :]
                        bc_ap = bass.AP(tensor=sl.tensor, offset=sl.offset,
                                        ap=[[0, 64]] + list(sl.ap))
                        nc.sync.dma_start(rb[:], bc_ap)
                        nc.vector.tensor_tensor(
                            outT[rows, hp, 512 * qc:512 * (qc + 1)],
                            po[0:64, :], rb[:], MULT)

            # ---- stage E: y = out @ W_proj (natural layout) ----
            wp_t = []
            for dc in range(NT):
                w = wpool.tile([128, D], F32R, tag="w")
                nc.sync.dma_start(w[:], wp_d[128 * dc:128 * (dc + 1), :].bitcast(F32R))
                wp_t.append(w)
            for st in range(ST):
                ps = psb.tile([128, 1024], F32, tag="big")
                for dc in range(NT):
                    nc.tensor.matmul(
                        ps[:, 0:512], outT[:, dc, 128 * st:128 * (st + 1)],
                        wp_t[dc][:, 0:512], start=(dc == 0), stop=(dc == NT - 1))
                for dc in range(NT):
                    nc.tensor.matmul(
                        ps[:, 512:768], outT[:, dc, 128 * st:128 * (st + 1)],
                        wp_t[dc][:, 512:768], start=(dc == 0), stop=(dc == NT - 1))
                y_sb = ypool.tile([128, D], F32, tag="y")
                nc.vector.tensor_copy(out=y_sb[:], in_=ps[:, 0:768])
                nc.sync.dma_start(y_d[128 * st:128 * (st + 1), :], y_sb[:])

    nc.compile()
    return nc


def _get_compiled():
    global _compiled
    if _compiled is None:
        _compiled = _build_nc()
    return _compiled


def kernel(x, W_attn, W_proj):
    from concourse.bass_utils import run_bass_kernel_spmd

    x = np.asarray(x, dtype=np.float32)
    W_attn = np.asarray(W_attn, dtype=np.float32)
    W_proj = np.asarray(W_proj, dtype=np.float32)

    xT = np.ascontiguousarray(np.transpose(x, (0, 2, 1)))  # [B, D, S]
    wq = np.ascontiguousarray(W_attn[:, 0:D]) * np.float32(0.125)
    wk = np.ascontiguousarray(W_attn[:, D:2 * D])
    wv = np.ascontiguousarray(W_attn[:, 2 * D:3 * D])
    masks = _build_masks()

    nc = _get_compiled()
    in_maps = [
        {"xT": xT[b], "wq": wq, "wk": wk, "wv": wv, "wp": W_proj, "masks": masks}
        for b in range(B)
    ]
    res = run_bass_kernel_spmd(nc, in_maps, list(range(NC_)))
    y = np.stack([res.results[b]["y"] for b in range(B)], axis=0)
    return y.astype(np.float32)


# revision 4
# speedup vs baseline: 1.1610x; 1.1610x over previous
"""Multi-head causal attention (B=8, S=1024, D=768, H=12) on 8 trn2 NeuronCores.

Strategy: data-parallel over batch (one batch element per core, no collectives).

Per-core dataflow (all matmuls fp32r except A@V in bf16):
  - host passes x^T, so Q^T/K^T come from a transposed projection
    (W stationary, x^T moving) and V from a natural projection
    (x^T stationary, W_v moving) -> no on-device transposes at all.
  - attention computed as S^T[k,q] = K @ Q^T per head, with two heads packed
    into the 128-row PE array via row tiling (dh=64).
  - softmax: exp on ScalarE straight out of PSUM (scale 1/8 folded into W_q
    host-side, no max-subtraction needed for these magnitudes); causal mask
    applied as a bf16 0/1 multiply on the few diagonal-crossing blocks;
    the denominator comes for free as row 64 of the A@V matmul by appending
    a ones column to V; division is folded into the PSUM->SBUF copy of the
    A@V result (reciprocal + DMA partition-broadcast via a DRAM scratch).
  - causal block-skip everywhere (upper-triangular blocks never computed).
"""
import sys

if "/opt/trn_rl_repo" not in sys.path:
    sys.path.insert(0, "/opt/trn_rl_repo")

import numpy as np

B, S, D, H = 8, 1024, 768, 12
DH = 64          # head dim
NC_ = 8          # cores
NT = D // 128    # 6 chunks of 128 along D
ST = S // 128    # 8 tiles of 128 along S
QC = S // 512    # 2 chunks of 512 along queries
VPW = H * (DH + 1)  # 780: v' row width (12 heads x (64 + ones col))

_compiled = None


def _build_masks():
    # mask[i, t, j] = 1 if (128*t + i) <= j else 0  (keep condition), bf16
    import ml_dtypes

    i = np.arange(128)[:, None, None]
    t = np.arange(4)[None, :, None]
    j = np.arange(512)[None, None, :]
    m = ((128 * t + i) <= j).astype(np.float32)
    return m.astype(ml_dtypes.bfloat16)


def _build_nc():
    import concourse.bass as bass
    import concourse.mybir as mybir
    import concourse.tile as tile
    from concourse import bacc

    F32 = mybir.dt.float32
    F32R = mybir.dt.float32r
    BF16 = mybir.dt.bfloat16
    AF = mybir.ActivationFunctionType
    MULT = mybir.AluOpType.mult

    nc = bacc.Bacc("TRN2", target_bir_lowering=False, debug=False)

    xT_d = nc.dram_tensor("xT", [D, S], F32, kind="ExternalInput")
    wq_d = nc.dram_tensor("wq", [D, D], F32, kind="ExternalInput")
    wk_d = nc.dram_tensor("wk", [D, D], F32, kind="ExternalInput")
    wv_d = nc.dram_tensor("wv", [D, D], F32, kind="ExternalInput")
    wp_d = nc.dram_tensor("wp", [D, D], F32, kind="ExternalInput")
    mask_d = nc.dram_tensor("masks", [128, 4, 512], BF16, kind="ExternalInput")
    y_d = nc.dram_tensor("y", [S, D], F32, kind="ExternalOutput")
    recip_d = nc.dram_tensor("recip_scratch", [H, QC, 512], F32)

    with tile.TileContext(nc) as tc:
        with (
            tc.tile_pool(name="static", bufs=1) as static,
            tc.tile_pool(name="w", bufs=13) as wpool,
            tc.tile_pool(name="pt", bufs=8) as ptpool,
            tc.tile_pool(name="small", bufs=2) as small,
            tc.tile_pool(name="rbp", bufs=3) as rbp,
            tc.tile_pool(name="y", bufs=2) as ypool,
            tc.tile_pool(name="psb", bufs=3, space="PSUM") as psb,
            tc.tile_pool(name="pso", bufs=2, space="PSUM") as pso,
        ):
            # ---- persistent SBUF ----
            xT = static.tile([128, NT, S], F32R)
            qT = static.tile([128, NT, S], F32R)
            kT = static.tile([128, NT, S], F32R)
            vp = static.tile([128, ST, VPW], BF16)
            outT = static.tile([128, NT, S], F32R)
            msk = static.tile([128, 4, 512], BF16)

            for dc in range(NT):
                nc.sync.dma_start(xT[:, dc, :], xT_d[128 * dc:128 * (dc + 1), :].bitcast(F32R))
            nc.sync.dma_start(msk[:], mask_d[:])
            nc.vector.memset(vp[:], 1.0)

            # ---- stage C: v' = x @ W_v (natural layout) + ones cols ----
            wv_t = []
            for dc in range(NT):
                w = wpool.tile([128, D], F32R, tag="w")
                nc.sync.dma_start(w[:], wv_d[128 * dc:128 * (dc + 1), :].bitcast(F32R))
                wv_t.append(w)
            for st in range(ST):
                ps = psb.tile([128, 1024], F32, tag="big")
                for dc in range(NT):
                    nc.tensor.matmul(
                        ps[:, 0:512], xT[:, dc, 128 * st:128 * (st + 1)],
                        wv_t[dc][:, 0:512], start=(dc == 0), stop=(dc == NT - 1))
                for dc in range(NT):
                    nc.tensor.matmul(
                        ps[:, 512:768], xT[:, dc, 128 * st:128 * (st + 1)],
                        wv_t[dc][:, 512:768], start=(dc == 0), stop=(dc == NT - 1))
                # scatter heads into v' (strided dest, ones cols preserved)
                dst = vp[:, st, :].rearrange("p (h e) -> p h e", e=DH + 1)
                nc.vector.tensor_copy(
                    out=dst[:, 0:8, 0:DH],
                    in_=ps[:, 0:512].rearrange("p (h d) -> p h d", d=DH))
                nc.vector.tensor_copy(
                    out=dst[:, 8:12, 0:DH],
                    in_=ps[:, 512:768].rearrange("p (h d) -> p h d", d=DH))

            # ---- interleaved: per head-pair hp: project qT/kT chunk, attention ----
            def project_chunk(w_tiles, nt, dst):
                # dst[:, nt, :] = (W[:, 128nt:128nt+128]).T @ xT   -> [128, 1024]
                ps = psb.tile([128, 1024], F32, tag="big")
                for dc in range(NT):
                    for sc in range(2):
                        nc.tensor.matmul(
                            ps[:, 512 * sc:512 * (sc + 1)],
                            w_tiles[dc][:, 128 * nt:128 * (nt + 1)],
                            xT[:, dc, 512 * sc:512 * (sc + 1)],
                            start=(dc == 0), stop=(dc == NT - 1))
                nc.vector.tensor_copy(out=dst[:, nt, :], in_=ps[:])

            wq_t, wk_t = [], []
            for dc in range(NT):
                w = wpool.tile([128, D], F32R, tag="w")
                nc.sync.dma_start(w[:], wq_d[128 * dc:128 * (dc + 1), :].bitcast(F32R))
                wq_t.append(w)
            for dc in range(NT):
                w = wpool.tile([128, D], F32R, tag="w")
                nc.sync.dma_start(w[:], wk_d[128 * dc:128 * (dc + 1), :].bitcast(F32R))
                wk_t.append(w)

            for hp in range(NT):  # head pair: heads 2hp (rows 0:64), 2hp+1 (rows 64:128)
                project_chunk(wq_t, hp, qT)
                project_chunk(wk_t, hp, kT)

                for qc in range(QC):
                    K = 4 * (qc + 1)  # causal: kc in [0, K)
                    pts = {0: [], 1: []}  # head_half -> list of pt tiles (kc pairs)
                    for kp in range(K // 2):
                        for hh in range(2):
                            rows = slice(64 * hh, 64 * (hh + 1))
                            ps = psb.tile([128, 1024], F32, tag="big")
                            for j in range(2):
                                kc = 2 * kp + j
                                nc.tensor.matmul(
                                    ps[:, 512 * j:512 * (j + 1)],
                                    kT[rows, hp, 128 * kc:128 * (kc + 1)],
                                    qT[rows, hp, 512 * qc:512 * (qc + 1)],
                                    start=True, stop=True,
                                    tile_position=(64 * hh, 0))
                            pt = ptpool.tile([128, 1024], BF16, tag="pt")
                            nc.scalar.activation(pt[:], ps[:], AF.Exp)
                            for j in range(2):
                                kc = 2 * kp + j
                                t = kc - 4 * qc
                                if 0 <= t <= 3:
                                    half = pt[:, 512 * j:512 * (j + 1)]
                                    nc.vector.tensor_tensor(half, half, msk[:, t, :], MULT)
                            pts[hh].append(pt)

                    for hh in range(2):
                        h = 2 * hp + hh
                        rows = slice(64 * hh, 64 * (hh + 1))
                        po = pso.tile([65, 512], F32, tag="po")
                        for kc in range(K):
                            nc.tensor.matmul(
                                po[:],
                                vp[:, kc, 65 * h:65 * (h + 1)],
                                pts[hh][kc // 2][:, 512 * (kc % 2):512 * (kc % 2 + 1)],
                                start=(kc == 0), stop=(kc == K - 1))
                        # denominator -> reciprocal -> broadcast -> normalize
                        den = small.tile([1, 512], F32, tag="den")
                        nc.vector.tensor_copy(out=den[:], in_=po[64:65, :])
                        rc = small.tile([1, 512], F32, tag="rc")
                        nc.vector.reciprocal_approx_fast(out=rc[:], in_=den[:])
                        nc.sync.dma_start(recip_d[h, qc, :], rc[:])
                        rb = rbp.tile([64, 512], F32, tag="rb")
                        sl = recip_d[h, qc, :]
                        bc_ap = bass.AP(tensor=sl.tensor, offset=sl.offset,
                                        ap=[[0, 64]] + list(sl.ap))
                        nc.sync.dma_start(rb[:], bc_ap)
                        nc.vector.tensor_tensor(
                            outT[rows, hp, 512 * qc:512 * (qc + 1)],
                            po[0:64, :], rb[:], MULT)

            # ---- stage E: y = out @ W_proj (natural layout) ----
            wp_t = []
            for dc in range(NT):
                w = wpool.tile([128, D], F32R, tag="w")
                nc.sync.dma_start(w[:], wp_d[128 * dc:128 * (dc + 1), :].bitcast(F32R))
                wp_t.append(w)
            for st in range(ST):
                ps = psb.tile([128, 1024], F32, tag="big")
                for dc in range(NT):
                    nc.tensor.matmul(
                        ps[:, 0:512], outT[:, dc, 128 * st:128 * (st + 1)],
                        wp_t[dc][:, 0:512], start=(dc == 0), stop=(dc == NT - 1))
                for dc in range(NT):
                    nc.tensor.matmul(
                        ps[:, 512:768], outT[:, dc, 128 * st:128 * (st + 1)],
                        wp_t[dc][:, 512:768], start=(dc == 0), stop=(dc == NT - 1))
                y_sb = ypool.tile([128, D], F32, tag="y")
                nc.vector.tensor_copy(out=y_sb[:], in_=ps[:, 0:768])
                nc.sync.dma_start(y_d[128 * st:128 * (st + 1), :], y_sb[:])

    nc.compile()
    return nc


def _get_compiled():
    global _compiled
    if _compiled is None:
        _compiled = _build_nc()
    return _compiled


def kernel(x, W_attn, W_proj):
    from concourse.bass_utils import run_bass_kernel_spmd

    x = np.asarray(x, dtype=np.float32)
    W_attn = np.asarray(W_attn, dtype=np.float32)
    W_proj = np.asarray(W_proj, dtype=np.float32)

    xT = np.ascontiguousarray(np.transpose(x, (0, 2, 1)))  # [B, D, S]
    wq = np.ascontiguousarray(W_attn[:, 0:D]) * np.float32(0.125)
    wk = np.ascontiguousarray(W_attn[:, D:2 * D])
    wv = np.ascontiguousarray(W_attn[:, 2 * D:3 * D])
    masks = _build_masks()

    nc = _get_compiled()
    in_maps = [
        {"xT": xT[b], "wq": wq, "wk": wk, "wv": wv, "wp": W_proj, "masks": masks}
        for b in range(B)
    ]
    res = run_bass_kernel_spmd(nc, in_maps, list(range(NC_)))
    y = np.stack([res.results[b]["y"] for b in range(B)], axis=0)
    return y.astype(np.float32)
